# revision 1
# baseline (speedup 1.0000x reference)
"""Causal self-attention Bass/Tile kernel for Trainium2, 8 NeuronCores (v3).

Sharding: batch (2) x head-groups (4 heads/core).  Host sums the 4 partial
output projections per batch.

Two-pass attention, no P transposes:
  pass 1 (stats): S[q,k] chunks in PSUM -> row max m (DVE), negated
  pass 2:        S^T[k,q] computed directly by a K=65 matmul where
                 row 64 of K^T is ones and row 64 of Q^T holds -m[q],
                 so PSUM already contains S^T - m.  ACT exp -> P^T fp16.
  diagonal causal masking via affine_select on P^T (zeros invalid k>q);
  AV matmul with [V | 1] fp16 appends the softmax denominator l as
  column 64; out rows scaled by 1/l during the PSUM->SBUF copy.
"""

import numpy as np

S = 2048
E = 1024
HPC = 4
D = 64
NCORES = 8
QB = S // 128     # 16 q-blocks
NSUP = S // 512   # 4 q-superblocks
KC = 8            # e chunks of 128
SCALE = 0.125     # 1/sqrt(64)

_CACHE = {}


def _build_nc():
    import concourse.bass as bass
    import concourse.mybir as mybir
    from concourse import tile

    f32 = mybir.dt.float32
    f16 = mybir.dt.float16
    X = mybir.AxisListType.X
    Exp = mybir.ActivationFunctionType.Exp

    nc = bass.Bass()

    x_d = nc.declare_dram_parameter("x", [S, E], f32, isOutput=False)
    wqk_d = nc.declare_dram_parameter("wqk", [E, HPC * 128], f32, isOutput=False)
    wv_d = nc.declare_dram_parameter("wv", [E, HPC * D], f32, isOutput=False)
    wo_d = nc.declare_dram_parameter("wo", [HPC * D, E], f32, isOutput=False)
    id_d = nc.declare_dram_parameter("ident", [128, 128], f32, isOutput=False)
    mask_d = nc.declare_dram_parameter("mask", [128, 128], f32, isOutput=False)
    out_d = nc.declare_dram_parameter("out", [S, E], f32, isOutput=True)

    with tile.TileContext(nc) as tc:
        with (
            tc.tile_pool(name="wpool", bufs=1) as wpool,
            tc.tile_pool(name="proj", bufs=1) as proj,
        ):
            # fused Q|K weights: wqk[:, c, h, 0:64] = W_q cols, [.., 64:128] = W_k
            wqk = wpool.tile([128, KC, HPC, 128], f32)
            wv = wpool.tile([128, KC, 256], f32)
            wo = wpool.tile([128, 2, E], f32)
            ident = wpool.tile([128, 128], f32)
            mask = wpool.tile([128, 128], f32)

            nc.sync.dma_start(
                wqk[:], wqk_d[:].rearrange("(c p) d -> p c d", p=128)
                .rearrange("p c (h e) -> p c h e", h=HPC))
            nc.sync.dma_start(wv[:], wv_d[:].rearrange("(c p) d -> p c d", p=128))
            nc.sync.dma_start(wo[:], wo_d[:].rearrange("(c p) e -> p c e", p=128))
            nc.sync.dma_start(ident[:], id_d[:])
            nc.sync.dma_start(mask[:], mask_d[:])

            # per-head QT/KT tiles [65, S]: rows 0-63 = projection (Q scaled),
            # row 64 of KT = ones, row 64 of QT = -m (written in phase C)
            qt = [proj.tile([65, S], f32, name=f"qt{h}", tag=f"qt{h}") for h in range(HPC)]
            kt = [proj.tile([65, S], f32, name=f"kt{h}", tag=f"kt{h}") for h in range(HPC)]
            vones = proj.tile([128, QB, HPC, D + 1], f16)

            # ---- phase A: x load + transpose (PE, via identity matmul) ----
            with tc.tile_pool(name="xtp", bufs=1) as xtp:
                xT = xtp.tile([128, KC, S], f32)
                with (
                    tc.tile_pool(name="xin", bufs=4) as xin,
                    tc.tile_pool(name="tps", bufs=2, space="PSUM") as tps,
                ):
                    for i in range(QB):
                        xt = xin.tile([128, E], f32)
                        nc.sync.dma_start(xt[:], x_d[128 * i:128 * (i + 1), :])
                        for g in range(2):
                            tp = tps.tile([128, 512], f32)
                            for t in range(4):
                                c = 4 * g + t
                                nc.tensor.matmul(
                                    tp[:, 128 * t:128 * (t + 1)],
                                    xt[:, 128 * c:128 * (c + 1)],
                                    ident[:],
                                )
                            dst = xT[:, 4 * g:4 * g + 4, 128 * i:128 * (i + 1)]
                            src = tp[:].rearrange("p (c s) -> p c s", c=4)
                            if g == 0:
                                nc.vector.tensor_copy(dst, src)
                            else:
                                nc.scalar.copy(dst, src)

                # ---- phase B: QKV projections ----
                with tc.tile_pool(name="qkv", bufs=4, space="PSUM") as qkv:
                    nc.vector.memset(vones[:, :, :, D:D + 1], 1.0)
                    for h in range(HPC):
                        nc.gpsimd.memset(kt[h][64:65, :], 1.0)
                    for h in range(HPC):
                        for sc in range(4):
                            pqk = qkv.tile([128, 512], f32, tag="qkv")
                            for c in range(KC):
                                nc.tensor.matmul(
                                    pqk[:], wqk[:, c, h, :],
                                    xT[:, c, 512 * sc:512 * (sc + 1)],
                                    start=(c == 0), stop=(c == KC - 1),
                                )
                            nc.scalar.mul(qt[h][0:D, 512 * sc:512 * (sc + 1)],
                                          pqk[0:D, :], SCALE)
                            nc.vector.tensor_copy(kt[h][0:D, 512 * sc:512 * (sc + 1)],
                                                  pqk[D:128, :])
                    for j in range(QB):
                        pv = qkv.tile([128, 256], f32, tag="qkv")
                        for c in range(KC):
                            nc.tensor.matmul(
                                pv[:], xT[:, c, 128 * j:128 * (j + 1)], wv[:, c, :],
                                start=(c == 0), stop=(c == KC - 1),
                            )
                        nc.scalar.copy(
                            vones[:, j, :, 0:D],
                            pv[:].rearrange("p (h d) -> p h d", h=HPC),
                        )

            # ---- phase C: attention ----
            attn = proj.tile([128, QB, HPC * D], f32)
            with (
                tc.tile_pool(name="sc", bufs=3, space="PSUM") as scp,
                tc.tile_pool(name="st", bufs=3, space="PSUM") as stp,
                tc.tile_pool(name="axp", bufs=2, space="PSUM") as axp,
                tc.tile_pool(name="pbuf", bufs=2) as pbuf,
                tc.tile_pool(name="stat", bufs=4) as stat,
            ):
                for h in range(HPC):
                    for sup in range(NSUP):
                        # --- pass 1: row-max stats for the 4 sub-blocks ---
                        negm4 = stat.tile([128, 4], f32, tag="n4")
                        for r in range(4):
                            i = 4 * sup + r
                            kn = 128 * (i + 1)
                            nch = (kn + 511) // 512
                            mparts = (stat.tile([128, 4], f32, tag="mp", name="mparts")
                                      if nch > 1 else None)
                            for c in range(nch):
                                n = min(512, kn - 512 * c)
                                sp = scp.tile([128, 512], f32, tag="sc")
                                nc.tensor.matmul(
                                    sp[:, :n],
                                    qt[h][0:D, 128 * i:128 * (i + 1)],
                                    kt[h][0:D, 512 * c:512 * c + n],
                                )
                                if c == nch - 1:
                                    # diagonal 128 cols: additive causal mask
                                    nc.vector.tensor_add(
                                        sp[:, n - 128:n], sp[:, n - 128:n], mask[:])
                                if nch == 1:
                                    nc.vector.reduce_max(negm4[:, r:r + 1], sp[:, :n],
                                                         axis=X, negate=True)
                                else:
                                    nc.vector.reduce_max(mparts[:, c:c + 1], sp[:, :n], axis=X)
                            if nch > 1:
                                nc.vector.reduce_max(negm4[:, r:r + 1], mparts[:, :nch],
                                                     axis=X, negate=True)
                        # --- transpose -m into qt row 64 ---
                        for r in range(4):
                            i = 4 * sup + r
                            nt = axp.tile([1, 128], f32, tag="ax", name="nt")
                            nc.tensor.matmul(nt[:], negm4[:, r:r + 1], ident[:])
                            dst = qt[h][64:65, 128 * i:128 * (i + 1)]
                            if r % 2 == 0:
                                nc.vector.tensor_copy(dst, nt[0:1, :])
                            else:
                                nc.scalar.copy(dst, nt[0:1, :])
                        # --- pass 2: S^T - m, exp -> P^T fp16 ---
                        pt = pbuf.tile([128, QB, 512], f16, tag="pt")
                        jmax = 4 * (sup + 1)
                        for j in range(jmax):
                            # causal: q-sub-blocks left of j's diagonal are
                            # invalid; skip them (stale pt there is zeroed by
                            # the affine_select below and never read by AV)
                            q0 = 128 * max(0, j - 4 * sup)
                            st = stp.tile([128, 512], f32, tag="st")
                            nc.tensor.matmul(
                                st[:, q0:],
                                kt[h][0:D + 1, 128 * j:128 * (j + 1)],
                                qt[h][0:D + 1, 512 * sup + q0:512 * (sup + 1)],
                            )
                            nc.scalar.activation(pt[:, j, q0:], st[:, q0:], Exp)
                        # --- causal mask on the 4 diagonal blocks ---
                        for r in range(4):
                            sl = pt[:, 4 * sup + r, 128 * r:]
                            nc.gpsimd.affine_select(
                                sl, sl,
                                pattern=[[1, 512 - 128 * r]],
                                compare_op=mybir.AluOpType.is_ge,
                                fill=0.0,
                                base=0,
                                channel_multiplier=-1,
                            )
                        # --- AV + denominator + normalize ---
                        for r in range(4):
                            i = 4 * sup + r
                            av = axp.tile([128, D + 1], f32, tag="ax")
                            for j in range(i + 1):
                                nc.tensor.matmul(
                                    av[:], pt[:, j, 128 * r:128 * (r + 1)],
                                    vones[:, j, h, :],
                                    start=(j == 0), stop=(j == i),
                                )
                            rl = stat.tile([128, 1], f32, tag="rl")
                            nc.vector.reciprocal(rl[:], av[:, D:D + 1])
                            nc.scalar.mul(attn[:, i, D * h:D * (h + 1)],
                                          av[:, 0:D], rl[:, 0:1])

            # ---- phase D: attn^T + output projection ----
            with (
                tc.tile_pool(name="aot", bufs=1) as aotp,
                tc.tile_pool(name="tps2", bufs=2, space="PSUM") as tps2,
                tc.tile_pool(name="ops", bufs=4, space="PSUM") as ops,
                tc.tile_pool(name="osb", bufs=3) as osb,
            ):
                aot = aotp.tile([128, 2, S], f32)
                for db in range(2):
                    for g in range(4):
                        tp = tps2.tile([128, 512], f32)
                        for t in range(4):
                            i = 4 * g + t
                            nc.tensor.matmul(
                                tp[:, 128 * t:128 * (t + 1)],
                                attn[:, i, 128 * db:128 * (db + 1)],
                                ident[:],
                            )
                        if g % 2 == 0:
                            nc.vector.tensor_copy(aot[:, db, 512 * g:512 * (g + 1)], tp[:])
                        else:
                            nc.scalar.copy(aot[:, db, 512 * g:512 * (g + 1)], tp[:])
                for sb in range(QB):
                    for ec in range(2):
                        po = ops.tile([128, 512], f32)
                        for kb in range(2):
                            nc.tensor.matmul(
                                po[:],
                                aot[:, kb, 128 * sb:128 * (sb + 1)],
                                wo[:, kb, 512 * ec:512 * (ec + 1)],
                                start=(kb == 0), stop=(kb == 1),
                            )
                        ob = osb.tile([128, 512], f32)
                        if ec == 0:
                            nc.scalar.copy(ob[:], po[:])
                        else:
                            nc.vector.tensor_copy(ob[:], po[:])
                        nc.sync.dma_start(
                            out_d[128 * sb:128 * (sb + 1), 512 * ec:512 * (ec + 1)],
                            ob[:],
                        )

    _split_excess_waits(nc)
    return nc


def _split_excess_waits(nc, maxw=1):
    """walrus here accepts one sync-wait per instruction; Tile's tail drain
    aggregates several.  Hoist excess waits onto preceding same-engine nops."""
    import concourse.mybir as mybir

    f = nc.m.functions[0]
    for b in f.blocks:
        insts = b.instructions
        i = 0
        while i < len(insts):
            inst = insts[i]
            si = inst.sync_info
            if si and si.on_wait and len(si.on_wait) > maxw:
                waits = list(si.on_wait)
                si.on_wait = waits[-maxw:]
                pos = i
                for w in waits[:-maxw]:
                    nop = nc.engines[inst.engine].nop(
                        nofuse=True, hint="wait_split"
                    ).ins
                    for bb in f.blocks:
                        L = bb.instructions
                        for k in range(len(L) - 1, -1, -1):
                            if L[k] is nop:
                                L.pop(k)
                                break
                    nsi = nop.sync_info
                    if nsi is None:
                        nop.sync_info = mybir.SyncInfo(on_wait=[w], on_update=[])
                    else:
                        nsi.on_wait = [w]
                    insts.insert(pos, nop)
                    pos += 1
                    i += 1
            i += 1


def _get_nc():
    if "nc" not in _CACHE:
        _CACHE["nc"] = _build_nc()
    return _CACHE["nc"]


def _make_in_maps(x, W_q, W_k, W_v, W_o):
    ident = np.eye(128, dtype=np.float32)
    r = np.arange(128)
    mask_np = np.where(r[None, :] <= r[:, None], 0.0, -1.0e30).astype(np.float32)
    in_maps = []
    for c in range(NCORES):
        b, g = c // 4, c % 4
        cs = slice(256 * g, 256 * (g + 1))
        wq_s = W_q[:, cs].reshape(E, HPC, D)
        wk_s = W_k[:, cs].reshape(E, HPC, D)
        wqk_s = np.concatenate([wq_s, wk_s], axis=2).reshape(E, HPC * 128)
        in_maps.append({
            "x": np.ascontiguousarray(x[b]),
            "wqk": np.ascontiguousarray(wqk_s),
            "mask": mask_np,
            "wv": np.ascontiguousarray(W_v[:, cs]),
            "wo": np.ascontiguousarray(W_o[cs, :]),
            "ident": ident,
        })
    return in_maps


def run_on_hw(x, W_q, W_k, W_v, W_o, trace=False):
    from concourse.bass_utils import run_bass_kernel_spmd

    nc = _get_nc()
    in_maps = _make_in_maps(x, W_q, W_k, W_v, W_o)
    res = run_bass_kernel_spmd(nc, in_maps, core_ids=list(range(NCORES)),
                               trace=trace)
    parts = [res.results[c]["out"] for c in range(NCORES)]
    out = np.stack([
        parts[0] + parts[1] + parts[2] + parts[3],
        parts[4] + parts[5] + parts[6] + parts[7],
    ]).astype(np.float32)
    return out, res


def kernel(x, W_q, W_k, W_v, W_o):
    x = np.asarray(x, dtype=np.float32)
    W_q = np.asarray(W_q, dtype=np.float32)
    W_k = np.asarray(W_k, dtype=np.float32)
    W_v = np.asarray(W_v, dtype=np.float32)
    W_o = np.asarray(W_o, dtype=np.float32)
    out, _ = run_on_hw(x, W_q, W_k, W_v, W_o, trace=False)
    return out

